# revision 20
# baseline (speedup 1.0000x reference)
"""Multi-head attention (b=8, c=512, t=1024, 8 heads, e=64) on 8 TRN2 cores.

Strategy: pure data-parallel over batch — each NeuronCore handles one batch
element; weights are replicated; no collectives.

The kernel is paced by the ScalarE exp stream (softmax numerators): 64 chunks
of [128 keys, 1024 cols] at ~1.1us each is the hard floor (~66us on ACT;
measured exp = (N+352)/1.2GHz cycles, dtype-independent, engine-unique).
Everything else is scheduled into the PE/DVE slack under that stream.

Per chunk (pair p, t-half th, key-block k):
  scores^T [128,1024] = k_h^T q_h   two heads row-packed via tile_position
                                    (0,0)/(64,0), ~310ns/concurrent pair
  P = exp(scores^T)                 ScalarE, the pacer; scores for the next
                                    chunk are emitted BEFORE exp(cur) so the
                                    stream never waits on the PE
  psav += v_h^T P                   col-packed head pair (0,0)/(0,64)
  psl  += ones^T P                  M=1 column sums; two chunks batched into
                                    one 4-way tile_position quad (cols
                                    0/32/64/96 concurrent, ~390ns for 4096
                                    cols); even/odd-chunk partial sums land
                                    in psl rows {0,32} vs {64,96} and are
                                    summed+broadcast by one E-matrix matmul
                                    at finalize (FWL-eligible full 128x128)
finalize(unit): lb=copy(psl); psbc = E^T lb (sum + broadcast l to 64 rows
per head); bc = 1/psbc; attnout = psav * bc  (all DVE except the matmul)

qkv/v/proj GEMMs are woven into per-slot PE slack as "fills" (qk fills split
into 2-matmul quanta); the projection's t-half-0 runs as late fills, t-half-1
in the tail. Input DMAs are host-prepacked into SBUF layout so each is one
contiguous 2D transfer; non-critical weights are gated behind x (tensor_copy
on gpsimd) so they don't steal HBM bandwidth from the startup-critical path.

Softmax max-subtraction is skipped (scores ~ N(0,1), exact in f32 psum).
The 1/sqrt(e) scale is folded into Wq/bq on host; the v-bias is folded into
the proj bias on host (y = Wp(AV/l + bv) + bp = Wp AV/l + (Wp bv + bp)).

PSUM budget (8 banks): scores 2x[128,1024] double-buffered (4) + psav
double-buffered (2) + psl (1) + fill/bcast scratch (1).
"""

import numpy as np
import ml_dtypes

_CACHE = {}

B, C, T = 8, 512, 1024
NH, E = 8, 64


def _build():
    import concourse.tile as tile
    from concourse import bacc, mybir

    F32 = mybir.dt.float32
    BF16 = mybir.dt.bfloat16
    AF = mybir.ActivationFunctionType

    nc = bacc.Bacc()

    # Host pre-packs x/weights into SBUF layout so every input DMA is one
    # contiguous 2D transfer (hardware-DGE fast path, large chunks):
    #   x_p[p, 1024c+t]            = x[128c+p, t]
    #   wq_p columns: [q0(4c) | k0(4c) | v(4c x 512) | q1 q2 q3 | k1 k2 k3]
    #   wp_p[p, 512o+128c+j]       = wprojT[128c+p, 128o+j]
    x_d = nc.declare_dram_parameter("x", [128, 4 * T], BF16, isOutput=False)
    wqkv_d = nc.declare_dram_parameter("wqkv", [128, 4 * 3 * C], BF16, isOutput=False)
    wproj_d = nc.declare_dram_parameter("wproj", [128, 4 * C], BF16, isOutput=False)
    bqk_d = nc.declare_dram_parameter("bqk", [128, 8], F32, isOutput=False)
    bproj_d = nc.declare_dram_parameter("bproj", [128, 4], F32, isOutput=False)
    out_d = nc.declare_dram_parameter("out", [C, T], BF16, isOutput=True)

    with tile.TileContext(nc) as tc:
        with (
            tc.tile_pool(name="persist", bufs=1) as per,
            tc.tile_pool(name="ppool", bufs=6) as pp,
            tc.tile_pool(name="scr", bufs=2) as scr,
            tc.tile_pool(name="psS", bufs=2, space="PSUM") as psS,
            tc.tile_pool(name="psA", bufs=2, space="PSUM") as psA,
            tc.tile_pool(name="psL", bufs=1, space="PSUM") as psL,
            tc.tile_pool(name="psF", bufs=1, space="PSUM") as psF,
        ):
            # ---- constants (no input deps) + PE warmup to release the HAM
            # clock gate while the runtime prologue / input DMAs run.
            ones = per.tile([128, 512], BF16, tag="ones")
            nc.vector.memset(ones[:], 1.0)
            # Broadcast matrix: psbc = Emat^T @ lb sums the even/odd-chunk
            # partial l rows {0,64} -> head-A cols 0-63, {32,96} -> head-B.
            Emat = per.tile([128, 128], BF16, tag="E")
            nc.gpsimd.memset(Emat[:], 0.0)
            nc.gpsimd.memset(Emat[0:1, 0:64], 1.0)
            nc.gpsimd.memset(Emat[64:65, 0:64], 1.0)
            nc.gpsimd.memset(Emat[32:33, 64:128], 1.0)
            nc.gpsimd.memset(Emat[96:97, 64:128], 1.0)
            for i in range(8):
                psw = psS.tile([128, 1024], F32, tag="S", name=f"warm{i}")
                nc.tensor.matmul(
                    psw[:, 0:512], lhsT=ones[:, 0:128], rhs=ones[:],
                    start=True, stop=True,
                )
            for i in range(6):  # fine-grained bridge keeps HAM warm until x lands
                psw = psS.tile([128, 1024], F32, tag="S", name=f"wb{i}")
                nc.tensor.matmul(
                    psw[:, 0:128], lhsT=ones[:, 0:128], rhs=ones[:, 0:128],
                    start=True, stop=True,
                )

            # ---- input DMAs, priority order: x, q/k cols for pair 0, wv,
            # bias, remaining qk cols, wproj. Spread across engine queues.
            xs_all = per.tile([128, 4 * T], BF16, tag="xs")
            wq_all = per.tile([128, 4 * 3 * C], BF16, tag="wqa")
            wp_all = per.tile([128, 4 * C], BF16, tag="wpa")
            xs = [xs_all[:, T * c : T * c + T] for c in range(4)]
            OFF_O = {0: 0, 4: 512, 1: 3072, 2: 3584, 3: 4096,
                     5: 4608, 6: 5120, 7: 5632}

            def wqo(c, o):  # stationary [128,128] block for qk GEMM (o, c)
                base = OFF_O[o] + 128 * c
                return wq_all[:, base : base + 128]

            def wv_sl(c):  # v-column block [128,512] for chunk c
                return wq_all[:, 1024 + 512 * c : 1024 + 512 * c + 512]

            def wpo(c, o):
                base = 512 * o + 128 * c
                return wp_all[:, base : base + 128]

            bqk = per.tile([128, 8], F32, tag="bqk")
            bpr = per.tile([128, 4], F32, tag="bpr")
            # priority: x, pair-0 q/k blocks, bias; gate the rest behind x.
            nc.sync.dma_start(out=xs_all[:], in_=x_d[:, :])
            nc.gpsimd.dma_start(out=wq_all[:, 0:512], in_=wqkv_d[:, 0:512])
            nc.scalar.dma_start(out=wq_all[:, 512:1024], in_=wqkv_d[:, 512:1024])
            nc.gpsimd.dma_start(out=bqk[:], in_=bqk_d[:, :])
            xgate = per.tile([1, 2], BF16, tag="xgate")
            nc.gpsimd.tensor_copy(xgate[:], xs_all[0:1, 0:2])
            nc.gpsimd.dma_start(out=wq_all[:, 1024:3072], in_=wqkv_d[:, 1024:3072])
            nc.gpsimd.dma_start(out=wq_all[:, 3072:4608], in_=wqkv_d[:, 3072:4608])
            nc.gpsimd.dma_start(out=wq_all[:, 4608:6144], in_=wqkv_d[:, 4608:6144])
            nc.gpsimd.dma_start(out=wp_all[:], in_=wproj_d[:, :])
            nc.gpsimd.dma_start(out=bpr[:], in_=bproj_d[:, :])

            # ---- persistent activations
            qk = [per.tile([128, T], BF16, tag=f"qk{o}", name=f"qk{o}") for o in range(8)]
            vT = [per.tile([128, C], BF16, tag=f"vT{t}", name=f"vT{t}") for t in range(8)]
            attnout = [per.tile([128, T], BF16, tag=f"ao{p}", name=f"ao{p}") for p in range(4)]
            out_sb = [per.tile([128, T], BF16, tag=f"os{o}", name=f"os{o}") for o in range(4)]

            def qk_lead_h0(o):  # lead-in: only the t-half 0 columns, via psS
                ps = psS.tile([128, 1024], F32, tag="S", name=f"qkl{o}")
                for c in range(4):
                    nc.tensor.matmul(
                        ps[:, 0:512],
                        lhsT=wqo(c, o),
                        rhs=xs[c][:, 0:512],
                        start=(c == 0), stop=(c == 3),
                    )
                nc.vector.tensor_scalar_add(
                    qk[o][:, 0:512], ps[:, 0:512], bqk[:, o : o + 1]
                )

            qkf_ps = {}

            def qk_fill_a(o, h):  # first half of the accumulation (c=0,1)
                ps = psF.tile([128, 512], F32, tag="f", name=f"qkf{o}_{h}")
                qkf_ps[(o, h)] = ps
                for c in range(2):
                    nc.tensor.matmul(
                        ps[:],
                        lhsT=wqo(c, o),
                        rhs=xs[c][:, 512 * h : 512 * h + 512],
                        start=(c == 0), stop=False,
                    )

            def qk_fill_b(o, h):  # second half (c=2,3) + bias to SBUF
                ps = qkf_ps.pop((o, h))
                for c in range(2, 4):
                    nc.tensor.matmul(
                        ps[:],
                        lhsT=wqo(c, o),
                        rhs=xs[c][:, 512 * h : 512 * h + 512],
                        start=False, stop=(c == 3),
                    )
                nc.vector.tensor_scalar_add(
                    qk[o][:, 512 * h : 512 * h + 512], ps[:], bqk[:, o : o + 1]
                )

            def qk_fill(o, h):
                qk_fill_a(o, h)
                qk_fill_b(o, h)

            def v_fill(tt):
                ps = psF.tile([128, 512], F32, tag="f", name=f"vf{tt}")
                for c in range(4):
                    nc.tensor.matmul(
                        ps[:],
                        lhsT=xs[c][:, 128 * tt : 128 * tt + 128],
                        rhs=wv_sl(c),
                        start=(c == 0), stop=(c == 3),
                    )
                nc.vector.tensor_copy(vT[tt][:], ps[:])

            units = [(p, th) for p in range(4) for th in range(2)]
            chunks = [
                (u, p, th, k)
                for u, (p, th) in enumerate(units)
                for k in range(8)
            ]

            sc = {}
            Ps = {}
            avt = {}
            psl_t = {}

            def scores(p, th, k):
                tsl = slice(512 * th, 512 * th + 512)
                ksl = slice(128 * k, 128 * k + 128)
                ps = psS.tile([128, 1024], F32, tag="S", name=f"S{p}_{th}_{k}")
                nc.tensor.matmul(
                    ps[:, 0:512],
                    lhsT=qk[4 + p][0:64, ksl], rhs=qk[p][0:64, tsl],
                    start=True, stop=True, tile_position=(0, 0),
                )
                nc.tensor.matmul(
                    ps[:, 512:1024],
                    lhsT=qk[4 + p][64:128, ksl], rhs=qk[p][64:128, tsl],
                    start=True, stop=True, tile_position=(64, 0),
                )
                return ps

            def av(u, p, k):
                if k == 0:
                    avt[u] = psA.tile([128, 512], F32, tag="av", name=f"av{u}")
                pa = avt[u]
                P = Ps[(u, k)]
                nc.tensor.matmul(
                    pa[0:64, :],
                    lhsT=vT[k][:, 128 * p : 128 * p + 64], rhs=P[:, 0:512],
                    start=(k == 0), stop=(k == 7), tile_position=(0, 0),
                )
                nc.tensor.matmul(
                    pa[64:128, :],
                    lhsT=vT[k][:, 128 * p + 64 : 128 * p + 128], rhs=P[:, 512:1024],
                    start=(k == 0), stop=(k == 7), tile_position=(0, 64),
                )

            def lquad(u, k1):  # column sums for chunks k1-1, k1 (k1 odd)
                q = k1 // 2
                if q == 0:
                    psl_t[u] = psL.tile([128, 512], F32, tag="l", name=f"l{u}")
                pl = psl_t[u]
                P0, P1 = Ps[(u, k1 - 1)], Ps[(u, k1)]
                st, sp = (q == 0), (q == 3)
                nc.tensor.matmul(pl[0:1, :], lhsT=ones[:, 0:1], rhs=P0[:, 0:512],
                                 start=st, stop=sp, tile_position=(0, 0))
                nc.tensor.matmul(pl[32:33, :], lhsT=ones[:, 0:1], rhs=P0[:, 512:1024],
                                 start=st, stop=sp, tile_position=(0, 32))
                nc.tensor.matmul(pl[64:65, :], lhsT=ones[:, 0:1], rhs=P1[:, 0:512],
                                 start=st, stop=sp, tile_position=(0, 64))
                nc.tensor.matmul(pl[96:97, :], lhsT=ones[:, 0:1], rhs=P1[:, 512:1024],
                                 start=st, stop=sp, tile_position=(0, 96))

            def fin_stage1(u):  # DVE only: move l partials out of psum
                lb = scr.tile([128, 512], BF16, tag="lb", name=f"lb{u}")
                nc.vector.tensor_copy(lb[:], psl_t[u][:])
                return lb

            def fin_stage2(u, lb):
                p, th = units[u]
                tsl = slice(512 * th, 512 * th + 512)
                ps = psF.tile([128, 512], F32, tag="f", name=f"bcm{u}")
                nc.tensor.matmul(ps[:], lhsT=Emat[:, 0:128], rhs=lb[:],
                                 start=True, stop=True)
                bc = scr.tile([128, 512], F32, tag="bc", name=f"rc{u}")
                nc.vector.reciprocal_approx_fast(bc[:], ps[:])
                nc.vector.tensor_mul(attnout[p][:, tsl], avt[u][:], bc[:])

            def proj_half0(o):
                ps = psF.tile([128, 512], F32, tag="f", name=f"pjf{o}")
                for c in range(4):
                    nc.tensor.matmul(
                        ps[:],
                        lhsT=wpo(c, o),
                        rhs=attnout[c][:, 0:512],
                        start=(c == 0), stop=(c == 3),
                    )
                nc.vector.tensor_scalar_add(
                    out_sb[o][:, 0:512], ps[:], bpr[:, o : o + 1]
                )
                eng = nc.sync if o % 2 == 0 else nc.gpsimd
                eng.dma_start(
                    out=out_d[128 * o : 128 * o + 128, 0:512],
                    in_=out_sb[o][:, 0:512],
                )

            # fills: loose deadlines exploited — scores chunk k reads only a
            # 128-col key slice, and th reads only one q half, so each unit
            # carries at most ~2 GEMM fills. Every fill is EMITTED before its
            # first reader (Tile tracks deps by program order).
            fills = {
                0: [lambda: qk_fill(4, 1)], 1: [lambda: v_fill(2)],
                2: [lambda: v_fill(3)], 3: [lambda: v_fill(4)],
                4: [lambda: qk_fill(0, 1)], 5: [lambda: v_fill(5)],
                6: [lambda: v_fill(6)], 7: [lambda: v_fill(7)],
                10: [lambda: qk_fill_a(1, 0)], 11: [lambda: qk_fill_b(1, 0)],
                12: [lambda: qk_fill_a(5, 0)], 13: [lambda: qk_fill_b(5, 0)],
                16: [lambda: qk_fill_a(5, 1)], 17: [lambda: qk_fill_b(5, 1)],
                18: [lambda: qk_fill_a(1, 1)], 19: [lambda: qk_fill_b(1, 1)],
                24: [lambda: qk_fill_a(2, 0)], 25: [lambda: qk_fill_b(2, 0)],
                26: [lambda: qk_fill_a(6, 0)], 27: [lambda: qk_fill_b(6, 0)],
                32: [lambda: qk_fill_a(6, 1)], 33: [lambda: qk_fill_b(6, 1)],
                34: [lambda: qk_fill_a(2, 1)], 35: [lambda: qk_fill_b(2, 1)],
                40: [lambda: qk_fill_a(3, 0)], 41: [lambda: qk_fill_b(3, 0)],
                42: [lambda: qk_fill_a(7, 0)], 43: [lambda: qk_fill_b(7, 0)],
                48: [lambda: qk_fill_a(7, 1)], 49: [lambda: qk_fill_b(7, 1)],
                50: [lambda: qk_fill_a(3, 1)], 51: [lambda: qk_fill_b(3, 1)],
                60: [lambda: proj_half0(0)], 62: [lambda: proj_half0(1)],
            }

            # ---- lead-in: t-half 0 of the first pair's q/k, first two v
            # tiles, first scores (the rest of qk[0]/qk[4] fills into unit 0)
            qk_lead_h0(0)
            qk_lead_h0(4)
            sc[(0, 0)] = scores(0, 0, 0)
            v_fill(0)
            v_fill(1)

            lbs = {}
            for i, (u, p, th, k) in enumerate(chunks):
                if i + 1 < len(chunks):
                    u2, p2, th2, k2 = chunks[i + 1]
                    sc[(u2, k2)] = scores(p2, th2, k2)
                P = pp.tile([128, 1024], BF16, tag="P", name=f"P{u}_{k}")
                nc.scalar.activation(P[:], sc.pop((u, k))[:], AF.Exp)
                Ps[(u, k)] = P
                if i >= 1:
                    u1, p1, th1, k1 = chunks[i - 1]
                    av(u1, p1, k1)
                if k == 4:
                    lquad(u, 1)
                elif k == 6:
                    lquad(u, 3)
                if u >= 1:
                    if k == 0:
                        lquad(u - 1, 5)
                    elif k == 1:
                        lquad(u - 1, 7)
                    elif k == 2:
                        lbs[u - 1] = fin_stage1(u - 1)
                    elif k == 3:
                        fin_stage2(u - 1, lbs.pop(u - 1))
                for f in fills.get(i, []):
                    f()

            # ---- tail: last chunk AV/l, final normalize (broadcast matmul
            # via the now-free psS pool so psF stays clear for proj fills),
            # remaining proj halves.
            # tail: start proj-th1 c=0..2 accumulation immediately (attnout
            # for c<=2 th1 is long ready; psS frees as the last exps retire),
            # overlap the finalize chain, then close with c=3 + bias + DMA.
            pj_ps = {}
            for o in (0, 1):
                ps = psS.tile([128, 1024], F32, tag="S", name=f"pj{o}")
                pj_ps[o] = ps
                for c in range(3):
                    nc.tensor.matmul(
                        ps[:, 0:512], lhsT=wpo(c, o),
                        rhs=attnout[c][:, 512:1024],
                        start=(c == 0), stop=False,
                    )
            u1, p1, th1, k1 = chunks[-1]
            av(u1, p1, k1)
            lquad(u1, 5)
            lquad(u1, 7)
            lbt = fin_stage1(u1)
            proj_half0(2)          # psF chain while fin runs on DVE
            psb = psF.tile([128, 512], F32, tag="f", name="bcm7")
            nc.tensor.matmul(psb[:], lhsT=Emat[:, 0:128], rhs=lbt[:],
                             start=True, stop=True)
            bc = scr.tile([128, 512], F32, tag="bc", name="rc7")
            nc.vector.reciprocal_approx_fast(bc[:], psb[:])
            nc.vector.tensor_mul(attnout[3][:, 512:1024], avt[u1][:], bc[:])

            def proj_th1_close(o):
                ps = pj_ps.pop(o)
                nc.tensor.matmul(
                    ps[:, 0:512], lhsT=wpo(3, o),
                    rhs=attnout[3][:, 512:1024],
                    start=False, stop=True,
                )
                nc.vector.tensor_scalar_add(
                    out_sb[o][:, 512:1024], ps[:, 0:512], bpr[:, o : o + 1]
                )
                eng = nc.sync if o % 2 == 0 else nc.gpsimd
                eng.dma_start(
                    out=out_d[128 * o : 128 * o + 128, 512:1024],
                    in_=out_sb[o][:, 512:1024],
                )

            proj_th1_close(0)
            proj_half0(3)
            proj_th1_close(1)
            for o in (2, 3):
                ps = psS.tile([128, 1024], F32, tag="S", name=f"pj{o}")
                for c in range(4):
                    nc.tensor.matmul(
                        ps[:, 0:512], lhsT=wpo(c, o),
                        rhs=attnout[c][:, 512:1024],
                        start=(c == 0), stop=(c == 3),
                    )
                nc.vector.tensor_scalar_add(
                    out_sb[o][:, 512:1024], ps[:, 0:512], bpr[:, o : o + 1]
                )
                eng = nc.sync if o % 2 == 0 else nc.gpsimd
                eng.dma_start(
                    out=out_d[128 * o : 128 * o + 128, 512:1024],
                    in_=out_sb[o][:, 512:1024],
                )

    nc.compile()
    return nc


def _get_nc():
    if "nc" not in _CACHE:
        _CACHE["nc"] = _build()
    return _CACHE["nc"]


def kernel(x, qkv_w, qkv_b, proj_w, proj_b, _trace=False):
    from concourse.bass_utils import run_bass_kernel_spmd

    nc = _get_nc()

    bf16 = ml_dtypes.bfloat16
    b, c, h, w = x.shape
    xf = np.asarray(x, dtype=np.float32).reshape(b, c, h * w)
    qkv_b = np.asarray(qkv_b, dtype=np.float32)
    qkv_w = np.asarray(qkv_w, dtype=np.float32)
    proj_w = np.asarray(proj_w, dtype=np.float32)
    proj_b = np.asarray(proj_b, dtype=np.float32)
    # fold the 1/sqrt(e)=1/8 softmax scale into Wq / bq on host
    qkv_w = np.concatenate([qkv_w[:512] * 0.125, qkv_w[512:]], axis=0)
    bq = np.concatenate([qkv_b[:512] * 0.125, qkv_b[512:1024]])
    # fold the v bias through the projection: y = Wp(AV/l + bv) + bp
    bproj_eff = proj_w @ qkv_b[1024:1536] + proj_b
    wqkvT = qkv_w.T.astype(np.float32)  # [c_in=512, 1536]
    wprojT = proj_w.T.astype(np.float32)  # [512, 512]
    OFF_O = {0: 0, 4: 512, 1: 3072, 2: 3584, 3: 4096, 5: 4608, 6: 5120, 7: 5632}
    wq_p = np.zeros((128, 6144), np.float32)
    for o in range(8):
        for cc in range(4):
            base = OFF_O[o] + 128 * cc
            wq_p[:, base : base + 128] = wqkvT[128 * cc : 128 * cc + 128,
                                               128 * o : 128 * o + 128]
    for cc in range(4):
        wq_p[:, 1024 + 512 * cc : 1024 + 512 * cc + 512] = wqkvT[
            128 * cc : 128 * cc + 128, 1024:1536
        ]
    wp_p = np.zeros((128, 2048), np.float32)
    for o in range(4):
        for cc in range(4):
            wp_p[:, 512 * o + 128 * cc : 512 * o + 128 * cc + 128] = wprojT[
                128 * cc : 128 * cc + 128, 128 * o : 128 * o + 128
            ]
    wq_p = np.ascontiguousarray(wq_p).astype(bf16)
    wp_p = np.ascontiguousarray(wp_p).astype(bf16)
    bqk = np.ascontiguousarray(bq.reshape(8, 128).T)
    bproj = np.ascontiguousarray(bproj_eff.reshape(4, 128).T)

    in_maps = [
        dict(
            x=np.ascontiguousarray(
                xf[i].reshape(4, 128, 1024).transpose(1, 0, 2).reshape(128, 4096)
            ).astype(bf16),
            wqkv=wq_p,
            wproj=wp_p,
            bqk=bqk,
            bproj=bproj,
        )
        for i in range(b)
    ]
    res = run_bass_kernel_spmd(nc, in_maps, core_ids=list(range(8)), trace=_trace)
    out = np.stack([res.results[i]["out"].astype(np.float32) for i in range(b)])
    out = out.reshape(b, c, h, w)
    if _trace:
        _CACHE["last_result"] = res
    return out


# revision 22
# speedup vs baseline: 1.0114x; 1.0114x over previous
"""Multi-head attention (b=8, c=512, t=1024, 8 heads, e=64) on 8 TRN2 cores.

Strategy: pure data-parallel over batch — each NeuronCore handles one batch
element; weights are replicated; no collectives.

The kernel is paced by the ScalarE exp stream (softmax numerators): 64 chunks
of [128 keys, 1024 cols] at ~1.1us each is the hard floor (~66us on ACT;
measured exp = (N+352)/1.2GHz cycles, dtype-independent, engine-unique).
Everything else is scheduled into the PE/DVE slack under that stream.

Per chunk (pair p, t-half th, key-block k):
  scores^T [128,1024] = k_h^T q_h   two heads row-packed via tile_position
                                    (0,0)/(64,0), ~310ns/concurrent pair
  P = exp(scores^T)                 ScalarE, the pacer; scores for the next
                                    chunk are emitted BEFORE exp(cur) so the
                                    stream never waits on the PE
  psav += v_h^T P                   col-packed head pair (0,0)/(0,64)
  psl  += ones^T P                  M=1 column sums; two chunks batched into
                                    one 4-way tile_position quad (cols
                                    0/32/64/96 concurrent, ~390ns for 4096
                                    cols); even/odd-chunk partial sums land
                                    in psl rows {0,32} vs {64,96} and are
                                    summed+broadcast by one E-matrix matmul
                                    at finalize (FWL-eligible full 128x128)
finalize(unit): lb=copy(psl); psbc = E^T lb (sum + broadcast l to 64 rows
per head); bc = 1/psbc; attnout = psav * bc  (all DVE except the matmul)

qkv/v/proj GEMMs are woven into per-slot PE slack as "fills" (qk fills split
into 2-matmul quanta); the projection's t-half-0 runs as late fills, t-half-1
in the tail. Input DMAs are host-prepacked into SBUF layout so each is one
contiguous 2D transfer; non-critical weights are gated behind x (tensor_copy
on gpsimd) so they don't steal HBM bandwidth from the startup-critical path.

Softmax max-subtraction is skipped (scores ~ N(0,1), exact in f32 psum).
The 1/sqrt(e) scale is folded into Wq/bq on host; the v-bias is folded into
the proj bias on host (y = Wp(AV/l + bv) + bp = Wp AV/l + (Wp bv + bp)).

PSUM budget (8 banks): scores 2x[128,1024] double-buffered (4) + psav
double-buffered (2) + psl (1) + fill/bcast scratch (1).
"""

import numpy as np
import ml_dtypes

_CACHE = {}

B, C, T = 8, 512, 1024
NH, E = 8, 64


def _build():
    import concourse.tile as tile
    from concourse import bacc, mybir

    F32 = mybir.dt.float32
    BF16 = mybir.dt.bfloat16
    AF = mybir.ActivationFunctionType

    nc = bacc.Bacc()

    # Host pre-packs x/weights into SBUF layout so every input DMA is one
    # contiguous 2D transfer (hardware-DGE fast path, large chunks):
    #   x_p[p, 1024c+t]            = x[128c+p, t]
    #   wq_p columns: [q0(4c) | k0(4c) | v(4c x 512) | q1 q2 q3 | k1 k2 k3]
    #   wp_p[p, 512o+128c+j]       = wprojT[128c+p, 128o+j]
    x_d = nc.declare_dram_parameter("x", [128, 4 * T], BF16, isOutput=False)
    wqkv_d = nc.declare_dram_parameter("wqkv", [128, 4 * 3 * C], BF16, isOutput=False)
    wproj_d = nc.declare_dram_parameter("wproj", [128, 4 * C], BF16, isOutput=False)
    bqk_d = nc.declare_dram_parameter("bqk", [128, 8], F32, isOutput=False)
    bproj_d = nc.declare_dram_parameter("bproj", [128, 4], F32, isOutput=False)
    out_d = nc.declare_dram_parameter("out", [C, T], BF16, isOutput=True)

    with tile.TileContext(nc) as tc:
        with (
            tc.tile_pool(name="persist", bufs=1) as per,
            tc.tile_pool(name="ppool", bufs=6) as pp,
            tc.tile_pool(name="scr", bufs=2) as scr,
            tc.tile_pool(name="psS", bufs=2, space="PSUM") as psS,
            tc.tile_pool(name="psA", bufs=2, space="PSUM") as psA,
            tc.tile_pool(name="psL", bufs=1, space="PSUM") as psL,
            tc.tile_pool(name="psF", bufs=1, space="PSUM") as psF,
        ):
            # ---- constants (no input deps) + PE warmup to release the HAM
            # clock gate while the runtime prologue / input DMAs run.
            ones = per.tile([128, 512], BF16, tag="ones")
            nc.vector.memset(ones[:], 1.0)
            # Broadcast matrix: psbc = Emat^T @ lb sums the even/odd-chunk
            # partial l rows {0,64} -> head-A cols 0-63, {32,96} -> head-B.
            Emat = per.tile([128, 128], BF16, tag="E")
            nc.gpsimd.memset(Emat[:], 0.0)
            nc.gpsimd.memset(Emat[0:1, 0:64], 1.0)
            nc.gpsimd.memset(Emat[64:65, 0:64], 1.0)
            nc.gpsimd.memset(Emat[32:33, 64:128], 1.0)
            nc.gpsimd.memset(Emat[96:97, 64:128], 1.0)
            for i in range(8):
                psw = psS.tile([128, 1024], F32, tag="S", name=f"warm{i}")
                nc.tensor.matmul(
                    psw[:, 0:512], lhsT=ones[:, 0:128], rhs=ones[:],
                    start=True, stop=True,
                )
            for i in range(6):  # fine-grained bridge keeps HAM warm until x lands
                psw = psS.tile([128, 1024], F32, tag="S", name=f"wb{i}")
                nc.tensor.matmul(
                    psw[:, 0:128], lhsT=ones[:, 0:128], rhs=ones[:, 0:128],
                    start=True, stop=True,
                )

            # ---- input DMAs, priority order: x, q/k cols for pair 0, wv,
            # bias, remaining qk cols, wproj. Spread across engine queues.
            xs_all = per.tile([128, 4 * T], BF16, tag="xs")
            wq_all = per.tile([128, 4 * 3 * C], BF16, tag="wqa")
            wp_all = per.tile([128, 4 * C], BF16, tag="wpa")
            xs = [xs_all[:, T * c : T * c + T] for c in range(4)]
            OFF_O = {0: 0, 4: 512, 1: 3072, 2: 3584, 3: 4096,
                     5: 4608, 6: 5120, 7: 5632}

            def wqo(c, o):  # stationary [128,128] block for qk GEMM (o, c)
                base = OFF_O[o] + 128 * c
                return wq_all[:, base : base + 128]

            def wv_sl(c):  # v-column block [128,512] for chunk c
                return wq_all[:, 1024 + 512 * c : 1024 + 512 * c + 512]

            def wpo(c, o):
                base = 512 * o + 128 * c
                return wp_all[:, base : base + 128]

            bqk = per.tile([128, 8], F32, tag="bqk")
            bpr = per.tile([128, 4], F32, tag="bpr")
            # priority: x, pair-0 q/k blocks, bias; gate the rest behind x.
            nc.sync.dma_start(out=xs_all[:, 0:2048], in_=x_d[:, 0:2048])
            nc.scalar.dma_start(out=xs_all[:, 2048:4096], in_=x_d[:, 2048:4096])
            nc.gpsimd.dma_start(out=wq_all[:, 0:512], in_=wqkv_d[:, 0:512])
            nc.sync.dma_start(out=wq_all[:, 512:1024], in_=wqkv_d[:, 512:1024])
            nc.gpsimd.dma_start(out=bqk[:], in_=bqk_d[:, :])
            # gate reads the two columns straddling the x split so it waits
            # for BOTH half-transfers before releasing the weight DMAs
            xgate = per.tile([1, 2], BF16, tag="xgate")
            nc.gpsimd.tensor_copy(xgate[:], xs_all[0:1, 2047:2049])
            nc.gpsimd.dma_start(out=wq_all[:, 1024:3072], in_=wqkv_d[:, 1024:3072])
            nc.gpsimd.dma_start(out=wq_all[:, 3072:4608], in_=wqkv_d[:, 3072:4608])
            nc.gpsimd.dma_start(out=wq_all[:, 4608:6144], in_=wqkv_d[:, 4608:6144])
            nc.gpsimd.dma_start(out=wp_all[:], in_=wproj_d[:, :])
            nc.gpsimd.dma_start(out=bpr[:], in_=bproj_d[:, :])

            # ---- persistent activations
            qk = [per.tile([128, T], BF16, tag=f"qk{o}", name=f"qk{o}") for o in range(8)]
            vT = [per.tile([128, C], BF16, tag=f"vT{t}", name=f"vT{t}") for t in range(8)]
            attnout = [per.tile([128, T], BF16, tag=f"ao{p}", name=f"ao{p}") for p in range(4)]
            out_sb = [per.tile([128, T], BF16, tag=f"os{o}", name=f"os{o}") for o in range(4)]

            def qk_lead_h0(o):  # lead-in: only the t-half 0 columns, via psS
                ps = psS.tile([128, 1024], F32, tag="S", name=f"qkl{o}")
                for c in range(4):
                    nc.tensor.matmul(
                        ps[:, 0:512],
                        lhsT=wqo(c, o),
                        rhs=xs[c][:, 0:512],
                        start=(c == 0), stop=(c == 3),
                    )
                nc.vector.tensor_scalar_add(
                    qk[o][:, 0:512], ps[:, 0:512], bqk[:, o : o + 1]
                )

            qkf_ps = {}

            def qk_fill_a(o, h):  # first half of the accumulation (c=0,1)
                ps = psF.tile([128, 512], F32, tag="f", name=f"qkf{o}_{h}")
                qkf_ps[(o, h)] = ps
                for c in range(2):
                    nc.tensor.matmul(
                        ps[:],
                        lhsT=wqo(c, o),
                        rhs=xs[c][:, 512 * h : 512 * h + 512],
                        start=(c == 0), stop=False,
                    )

            def qk_fill_b(o, h):  # second half (c=2,3) + bias to SBUF
                ps = qkf_ps.pop((o, h))
                for c in range(2, 4):
                    nc.tensor.matmul(
                        ps[:],
                        lhsT=wqo(c, o),
                        rhs=xs[c][:, 512 * h : 512 * h + 512],
                        start=False, stop=(c == 3),
                    )
                nc.vector.tensor_scalar_add(
                    qk[o][:, 512 * h : 512 * h + 512], ps[:], bqk[:, o : o + 1]
                )

            def qk_fill(o, h):
                qk_fill_a(o, h)
                qk_fill_b(o, h)

            def v_fill(tt):
                ps = psF.tile([128, 512], F32, tag="f", name=f"vf{tt}")
                for c in range(4):
                    nc.tensor.matmul(
                        ps[:],
                        lhsT=xs[c][:, 128 * tt : 128 * tt + 128],
                        rhs=wv_sl(c),
                        start=(c == 0), stop=(c == 3),
                    )
                nc.vector.tensor_copy(vT[tt][:], ps[:])

            units = [(p, th) for p in range(4) for th in range(2)]
            chunks = [
                (u, p, th, k)
                for u, (p, th) in enumerate(units)
                for k in range(8)
            ]

            sc = {}
            Ps = {}
            avt = {}
            psl_t = {}

            def scores(p, th, k):
                tsl = slice(512 * th, 512 * th + 512)
                ksl = slice(128 * k, 128 * k + 128)
                ps = psS.tile([128, 1024], F32, tag="S", name=f"S{p}_{th}_{k}")
                nc.tensor.matmul(
                    ps[:, 0:512],
                    lhsT=qk[4 + p][0:64, ksl], rhs=qk[p][0:64, tsl],
                    start=True, stop=True, tile_position=(0, 0),
                )
                nc.tensor.matmul(
                    ps[:, 512:1024],
                    lhsT=qk[4 + p][64:128, ksl], rhs=qk[p][64:128, tsl],
                    start=True, stop=True, tile_position=(64, 0),
                )
                return ps

            def av(u, p, k):
                if k == 0:
                    avt[u] = psA.tile([128, 512], F32, tag="av", name=f"av{u}")
                pa = avt[u]
                P = Ps[(u, k)]
                nc.tensor.matmul(
                    pa[0:64, :],
                    lhsT=vT[k][:, 128 * p : 128 * p + 64], rhs=P[:, 0:512],
                    start=(k == 0), stop=(k == 7), tile_position=(0, 0),
                )
                nc.tensor.matmul(
                    pa[64:128, :],
                    lhsT=vT[k][:, 128 * p + 64 : 128 * p + 128], rhs=P[:, 512:1024],
                    start=(k == 0), stop=(k == 7), tile_position=(0, 64),
                )

            def lquad(u, k1):  # column sums for chunks k1-1, k1 (k1 odd)
                q = k1 // 2
                if q == 0:
                    psl_t[u] = psL.tile([128, 512], F32, tag="l", name=f"l{u}")
                pl = psl_t[u]
                P0, P1 = Ps[(u, k1 - 1)], Ps[(u, k1)]
                st, sp = (q == 0), (q == 3)
                nc.tensor.matmul(pl[0:1, :], lhsT=ones[:, 0:1], rhs=P0[:, 0:512],
                                 start=st, stop=sp, tile_position=(0, 0))
                nc.tensor.matmul(pl[32:33, :], lhsT=ones[:, 0:1], rhs=P0[:, 512:1024],
                                 start=st, stop=sp, tile_position=(0, 32))
                nc.tensor.matmul(pl[64:65, :], lhsT=ones[:, 0:1], rhs=P1[:, 0:512],
                                 start=st, stop=sp, tile_position=(0, 64))
                nc.tensor.matmul(pl[96:97, :], lhsT=ones[:, 0:1], rhs=P1[:, 512:1024],
                                 start=st, stop=sp, tile_position=(0, 96))

            def fin_stage1(u):  # DVE only: move l partials out of psum
                lb = scr.tile([128, 512], BF16, tag="lb", name=f"lb{u}")
                nc.vector.tensor_copy(lb[:], psl_t[u][:])
                return lb

            def fin_stage2(u, lb):
                p, th = units[u]
                tsl = slice(512 * th, 512 * th + 512)
                ps = psF.tile([128, 512], F32, tag="f", name=f"bcm{u}")
                nc.tensor.matmul(ps[:], lhsT=Emat[:, 0:128], rhs=lb[:],
                                 start=True, stop=True)
                bc = scr.tile([128, 512], F32, tag="bc", name=f"rc{u}")
                nc.vector.reciprocal_approx_fast(bc[:], ps[:])
                nc.vector.tensor_mul(attnout[p][:, tsl], avt[u][:], bc[:])

            def proj_half0(o):
                ps = psF.tile([128, 512], F32, tag="f", name=f"pjf{o}")
                for c in range(4):
                    nc.tensor.matmul(
                        ps[:],
                        lhsT=wpo(c, o),
                        rhs=attnout[c][:, 0:512],
                        start=(c == 0), stop=(c == 3),
                    )
                nc.vector.tensor_scalar_add(
                    out_sb[o][:, 0:512], ps[:], bpr[:, o : o + 1]
                )
                eng = nc.sync if o % 2 == 0 else nc.gpsimd
                eng.dma_start(
                    out=out_d[128 * o : 128 * o + 128, 0:512],
                    in_=out_sb[o][:, 0:512],
                )

            # fills: loose deadlines exploited — scores chunk k reads only a
            # 128-col key slice, and th reads only one q half, so each unit
            # carries at most ~2 GEMM fills. Every fill is EMITTED before its
            # first reader (Tile tracks deps by program order).
            fills = {
                0: [lambda: qk_fill(4, 1)], 1: [lambda: v_fill(2)],
                2: [lambda: v_fill(3)], 3: [lambda: v_fill(4)],
                4: [lambda: qk_fill(0, 1)], 5: [lambda: v_fill(5)],
                6: [lambda: v_fill(6)], 7: [lambda: v_fill(7)],
                10: [lambda: qk_fill_a(1, 0)], 11: [lambda: qk_fill_b(1, 0)],
                12: [lambda: qk_fill_a(5, 0)], 13: [lambda: qk_fill_b(5, 0)],
                16: [lambda: qk_fill_a(5, 1)], 17: [lambda: qk_fill_b(5, 1)],
                18: [lambda: qk_fill_a(1, 1)], 19: [lambda: qk_fill_b(1, 1)],
                24: [lambda: qk_fill_a(2, 0)], 25: [lambda: qk_fill_b(2, 0)],
                26: [lambda: qk_fill_a(6, 0)], 27: [lambda: qk_fill_b(6, 0)],
                32: [lambda: qk_fill_a(6, 1)], 33: [lambda: qk_fill_b(6, 1)],
                34: [lambda: qk_fill_a(2, 1)], 35: [lambda: qk_fill_b(2, 1)],
                40: [lambda: qk_fill_a(3, 0)], 41: [lambda: qk_fill_b(3, 0)],
                42: [lambda: qk_fill_a(7, 0)], 43: [lambda: qk_fill_b(7, 0)],
                48: [lambda: qk_fill_a(7, 1)], 49: [lambda: qk_fill_b(7, 1)],
                50: [lambda: qk_fill_a(3, 1)], 51: [lambda: qk_fill_b(3, 1)],
                60: [lambda: proj_half0(0)], 62: [lambda: proj_half0(1)],
            }

            # ---- lead-in: t-half 0 of the first pair's q/k, first two v
            # tiles, first scores (the rest of qk[0]/qk[4] fills into unit 0)
            qk_lead_h0(0)
            qk_lead_h0(4)
            sc[(0, 0)] = scores(0, 0, 0)
            v_fill(0)
            v_fill(1)

            lbs = {}
            for i, (u, p, th, k) in enumerate(chunks):
                if i + 1 < len(chunks):
                    u2, p2, th2, k2 = chunks[i + 1]
                    sc[(u2, k2)] = scores(p2, th2, k2)
                P = pp.tile([128, 1024], BF16, tag="P", name=f"P{u}_{k}")
                nc.scalar.activation(P[:], sc.pop((u, k))[:], AF.Exp)
                Ps[(u, k)] = P
                if i >= 1:
                    u1, p1, th1, k1 = chunks[i - 1]
                    av(u1, p1, k1)
                if k == 4:
                    lquad(u, 1)
                elif k == 6:
                    lquad(u, 3)
                if u >= 1:
                    if k == 0:
                        lquad(u - 1, 5)
                    elif k == 1:
                        lquad(u - 1, 7)
                    elif k == 2:
                        lbs[u - 1] = fin_stage1(u - 1)
                    elif k == 3:
                        fin_stage2(u - 1, lbs.pop(u - 1))
                for f in fills.get(i, []):
                    f()

            # ---- tail: last chunk AV/l, final normalize (broadcast matmul
            # via the now-free psS pool so psF stays clear for proj fills),
            # remaining proj halves.
            u1, p1, th1, k1 = chunks[-1]
            av(u1, p1, k1)
            lquad(u1, 5)
            lquad(u1, 7)
            lbt = fin_stage1(u1)
            proj_half0(2)          # psF chain while fin runs on DVE
            psb = psS.tile([128, 1024], F32, tag="S", name="bcm7")
            nc.tensor.matmul(psb[:, 0:512], lhsT=Emat[:, 0:128], rhs=lbt[:],
                             start=True, stop=True)
            bc = scr.tile([128, 512], F32, tag="bc", name="rc7")
            nc.vector.reciprocal_approx_fast(bc[:], psb[:, 0:512])
            nc.vector.tensor_mul(attnout[3][:, 512:1024], avt[u1][:], bc[:])
            proj_half0(3)

            def proj_th1(o, pool):
                if pool is psS:
                    ps = pool.tile([128, 1024], F32, tag="S", name=f"pj{o}")[:, 0:512]
                else:
                    ps = pool.tile([128, 512], F32, tag="f", name=f"pj{o}")[:]
                for c in range(4):
                    nc.tensor.matmul(
                        ps,
                        lhsT=wpo(c, o),
                        rhs=attnout[c][:, 512:1024],
                        start=(c == 0), stop=(c == 3),
                    )
                nc.vector.tensor_scalar_add(
                    out_sb[o][:, 512:1024], ps, bpr[:, o : o + 1]
                )
                eng = nc.sync if o % 2 == 0 else nc.gpsimd
                eng.dma_start(
                    out=out_d[128 * o : 128 * o + 128, 512:1024],
                    in_=out_sb[o][:, 512:1024],
                )

            proj_th1(0, psS)
            proj_th1(1, psF)
            proj_th1(2, psS)
            proj_th1(3, psF)

    nc.compile()
    return nc


def _get_nc():
    if "nc" not in _CACHE:
        _CACHE["nc"] = _build()
    return _CACHE["nc"]


def kernel(x, qkv_w, qkv_b, proj_w, proj_b, _trace=False):
    from concourse.bass_utils import run_bass_kernel_spmd

    nc = _get_nc()

    bf16 = ml_dtypes.bfloat16
    b, c, h, w = x.shape
    xf = np.asarray(x, dtype=np.float32).reshape(b, c, h * w)
    qkv_b = np.asarray(qkv_b, dtype=np.float32)
    qkv_w = np.asarray(qkv_w, dtype=np.float32)
    proj_w = np.asarray(proj_w, dtype=np.float32)
    proj_b = np.asarray(proj_b, dtype=np.float32)
    # fold the 1/sqrt(e)=1/8 softmax scale into Wq / bq on host
    qkv_w = np.concatenate([qkv_w[:512] * 0.125, qkv_w[512:]], axis=0)
    bq = np.concatenate([qkv_b[:512] * 0.125, qkv_b[512:1024]])
    # fold the v bias through the projection: y = Wp(AV/l + bv) + bp
    bproj_eff = proj_w @ qkv_b[1024:1536] + proj_b
    wqkvT = qkv_w.T.astype(np.float32)  # [c_in=512, 1536]
    wprojT = proj_w.T.astype(np.float32)  # [512, 512]
    OFF_O = {0: 0, 4: 512, 1: 3072, 2: 3584, 3: 4096, 5: 4608, 6: 5120, 7: 5632}
    wq_p = np.zeros((128, 6144), np.float32)
    for o in range(8):
        for cc in range(4):
            base = OFF_O[o] + 128 * cc
            wq_p[:, base : base + 128] = wqkvT[128 * cc : 128 * cc + 128,
                                               128 * o : 128 * o + 128]
    for cc in range(4):
        wq_p[:, 1024 + 512 * cc : 1024 + 512 * cc + 512] = wqkvT[
            128 * cc : 128 * cc + 128, 1024:1536
        ]
    wp_p = np.zeros((128, 2048), np.float32)
    for o in range(4):
        for cc in range(4):
            wp_p[:, 512 * o + 128 * cc : 512 * o + 128 * cc + 128] = wprojT[
                128 * cc : 128 * cc + 128, 128 * o : 128 * o + 128
            ]
    wq_p = np.ascontiguousarray(wq_p).astype(bf16)
    wp_p = np.ascontiguousarray(wp_p).astype(bf16)
    bqk = np.ascontiguousarray(bq.reshape(8, 128).T)
    bproj = np.ascontiguousarray(bproj_eff.reshape(4, 128).T)

    in_maps = [
        dict(
            x=np.ascontiguousarray(
                xf[i].reshape(4, 128, 1024).transpose(1, 0, 2).reshape(128, 4096)
            ).astype(bf16),
            wqkv=wq_p,
            wproj=wp_p,
            bqk=bqk,
            bproj=bproj,
        )
        for i in range(b)
    ]
    res = run_bass_kernel_spmd(nc, in_maps, core_ids=list(range(8)), trace=_trace)
    out = np.stack([res.results[i]["out"].astype(np.float32) for i in range(b)])
    out = out.reshape(b, c, h, w)
    if _trace:
        _CACHE["last_result"] = res
    return out
